# revision 10
# baseline (speedup 1.0000x reference)
"""DKEPooling Trainium2 kernel, v2 (all-bf16, stage-pipelined).

Per-graph math (d=256, n=512 nodes/graph):
  g  = bf16(feat + 0.01*noise)
  C' = g^T g - s s^T/n            (Gram + rank-1, PSUM accumulated)
  tr = trace(C'); A = C'/tr       (A never materialized: 1/tr folded into evacs)
  V_k' := 1.5I - 0.5 T_k ;  Q_k' = T_k V_k' ;  T_{k+1} = Q_k' V_k'
    (== Newton-Schulz invariant T_{k+1} = 0.25 T_k (3I-T_k)^2, T_0 = A;
     Q_0' = 0.5*A(3I-A) = 0.5*W1 doubles as the final NS factor)
  y  = sqrt(tr/(n-1)) * (1/32) * W1 (3I-T1)(3I-T2)(3I-T3)(3I-T4) * s/n
    tail: 7 matvec steps/graph; (3I-T4)v0 via the 3-matvec Krylov trick;
    T3 is never materialized - the chain stops at Q2/V2' and the tail
    applies T3 v = Q2 (V2' v) as two chained column matvecs.

Scheduling: software pipeline with 9 stages skewed across the 16 graphs so
PE/ACT/DVE streams stay dense (engines execute in program order; emission
order is the schedule); PSUM evacuations are load-balanced between ACT and
DVE (DVE PSUM reads work on this runtime).  Tail (4-graph waves, ticks
interleaved into late phase-A): matvecs run column-form - stationary = X
block, moving = v column - so u lands directly as PSUM columns and the
combine (3v - u) reads PSUM on DVE; only v0 needs a row->column transpose
(bf16 quadrant rows + E4 selector matmul) and only the final step uses
row-form for the output DMA.  No DMAs inside tail steps.

Sharding: data-parallel over graphs. 8 cores x 16 graphs; no cross-core comm.
"""
import numpy as np

import concourse.bacc as bacc
import concourse.bass as bass
import concourse.mybir as mybir
import concourse.tile as tile
from concourse.bass_utils import run_bass_kernel_spmd

F32 = mybir.dt.float32
BF16 = mybir.dt.bfloat16
ALU = mybir.AluOpType
ACTF = mybir.ActivationFunctionType

N_CORES = 8
D = 256
NPG = 512
B_TOTAL = 128
B_CORE = B_TOTAL // N_CORES      # 16 graphs per core
ROWS_CORE = B_CORE * NPG         # 8192 feat rows per core
WAVE = 4                         # graphs per tail wave
N_WAVES = B_CORE // WAVE

# const tensor (bf16 [128, 646]):
#   [:, 0:512]   C15: 1.5*I_256 in block layout ([1.5I|0 | 0|1.5I])
#                ([:, 128:256] doubles as a zero block)
#   [:, 512:640] I128
#   [:, 640:644] E4 selector (E4[32q, q] = 1)
#   [:, 644:645] ones column
CST_COLS = 646

DEBUG_HOOK = None
MID_HOOK = None


def _const_arrays():
    import ml_dtypes
    cst = np.zeros((128, CST_COLS), np.float32)
    eye = np.eye(128, dtype=np.float32)
    cst[:, 0:128] = 1.5 * eye
    cst[:, 384:512] = 1.5 * eye
    cst[:, 512:640] = eye
    for q in range(4):
        cst[32 * q, 640 + q] = 1.0
    cst[:, 644] = 1.0
    return cst.astype(ml_dtypes.bfloat16)


def build_module():
    nc = bacc.Bacc(None, target_bir_lowering=False)
    feat_d = nc.declare_dram_parameter("feat", [ROWS_CORE, D], F32, isOutput=False)
    noise_d = nc.declare_dram_parameter("noise", [ROWS_CORE, D], F32, isOutput=False)
    cst_d = nc.declare_dram_parameter("cst", [128, CST_COLS], BF16, isOutput=False)
    out_d = nc.declare_dram_parameter("out", [B_CORE, D], F32, isOutput=True)

    with tile.TileContext(nc) as tc:
        _build_tile(tc, nc, feat_d, noise_d, cst_d, out_d)
    nc.compile()
    return nc


def _build_tile(tc, nc, feat_d, noise_d, cst_d, out_d):
    import contextlib
    import concourse.bass_isa as bass_isa
    ctx = contextlib.ExitStack()
    with ctx:
        stage_p = ctx.enter_context(tc.tile_pool(name="stage", bufs=2))
        g_p = ctx.enter_context(tc.tile_pool(name="gp", bufs=3))
        mats_p = ctx.enter_context(tc.tile_pool(name="mats", bufs=B_CORE))
        chain_p = ctx.enter_context(tc.tile_pool(name="chain", bufs=4))
        small_p = ctx.enter_context(tc.tile_pool(name="small", bufs=4))
        tail_p = ctx.enter_context(tc.tile_pool(name="tailp", bufs=4))
        vcol_p = ctx.enter_context(tc.tile_pool(name="vcolp", bufs=12))
        cst_p = ctx.enter_context(tc.tile_pool(name="cstp", bufs=1))
        psG = ctx.enter_context(tc.tile_pool(name="psG", bufs=2, space="PSUM"))
        psC = ctx.enter_context(tc.tile_pool(name="psC", bufs=2, space="PSUM"))
        psS = ctx.enter_context(tc.tile_pool(name="psS", bufs=1, space="PSUM"))
        psR = ctx.enter_context(tc.tile_pool(name="psR", bufs=1, space="PSUM"))
        psV = ctx.enter_context(tc.tile_pool(name="psV", bufs=2, space="PSUM"))

        cst = cst_p.tile([128, CST_COLS], BF16, tag="cst", name="cst_sb")
        nc.gpsimd.dma_start(cst, cst_d[:, :])
        C15 = cst[:, 0:512]
        ZBLK = cst[:, 128:256]
        I128 = cst[:, 512:640]
        E4 = cst[:, 640:644]
        ONES = cst[:, 644:645]

        v0rows = []
        for w in range(N_WAVES):
            vr = tail_p.tile([128, 256], BF16, tag="v0rows", name=f"v0rows_{w}")
            nc.vector.memset(vr, 0.0)
            v0rows.append(vr)

        # per-graph state kept across pipeline stages
        ST = [dict() for _ in range(B_CORE)]

        def s_load(g):
            # split across queues: feat on SP HWDGE, noise on gpsimd SWDGE
            st = ST[g]
            ft = stage_p.tile([128, 4 * D], F32, tag="ft", name=f"ft_{g}")
            nc.sync.dma_start(
                ft, feat_d[g * NPG:(g + 1) * NPG, :].rearrange("(c p) d -> p c d", p=128))
            nz = stage_p.tile([128, 4 * D], F32, tag="nz", name=f"nz_{g}")
            nc.gpsimd.dma_start(
                nz, noise_d[g * NPG:(g + 1) * NPG, :].rearrange("(c p) d -> p c d", p=128))
            st["ft"], st["nz"] = ft, nz

        def s_gb(g):
            st = ST[g]
            gb = g_p.tile([128, 4 * D], BF16, tag="g", name=f"g_{g}")
            nc.vector.scalar_tensor_tensor(gb, st["nz"], 0.01, st["ft"],
                                           ALU.mult, ALU.add)
            st["gb"] = gb

        def s_scol(g):
            st = ST[g]
            gb = st["gb"]
            s_ps = psS.tile([1, D], F32, tag="s", name=f"s_{g}")
            for k in range(4):
                nc.tensor.matmul(s_ps, ONES, gb[:, 256 * k:256 * (k + 1)],
                                 start=(k == 0), stop=(k == 3))
            srow = small_p.tile([1, D], BF16, tag="srow", name=f"srow_{g}")
            nc.scalar.copy(srow, s_ps)
            srow_n = small_p.tile([1, D], BF16, tag="srow_n", name=f"srown_{g}")
            nc.vector.tensor_scalar_mul(srow_n, srow, -1.0 / NPG)
            st["srow"], st["srow_n"] = srow, srow_n

        def s_gram(g):
            # PSUM accumulation groups must be contiguous per bank: emit each
            # m-half's full group (4 gram chunks + rank-1 mean correction)
            # before opening the other half's group.
            st = ST[g]
            gb = st["gb"]
            srow, srow_n = st["srow"], st["srow_n"]
            G = psG.tile([128, 512], F32, tag="G", name=f"G_{g}")
            for m in range(2):
                for k in range(4):
                    nc.tensor.matmul(G[:, 256 * m:256 * (m + 1)],
                                     gb[:, 256 * k + 128 * m:256 * k + 128 * (m + 1)],
                                     gb[:, 256 * k:256 * (k + 1)],
                                     start=(k == 0), stop=False)
                nc.tensor.matmul(G[:, 256 * m:256 * (m + 1)],
                                 srow_n[0:1, 128 * m:128 * (m + 1)], srow,
                                 start=False, stop=True)
            st["G"] = G

        def s_mid(g):
            st = ST[g]
            G = st["G"]
            Gc = chain_p.tile([128, 512], BF16, tag="Gc", name=f"Gc_{g}")
            nc.scalar.copy(Gc, G)
            if MID_HOOK is not None:
                MID_HOOK(g, nc, Gc, G)
            # trace via diag mask + partition all-reduce, straight from PSUM
            # (runs in parallel with the Gc evacuation)
            dg = small_p.tile([128, 2], F32, tag="dg", name=f"dg_{g}")
            for m in range(2):
                scr = small_p.tile([128, 128], BF16, tag="scr", name=f"scr_{g}_{m}")
                nc.vector.scalar_tensor_tensor(scr, G[:, 384 * m:384 * m + 128],
                                               1.0, I128, ALU.mult, ALU.mult,
                                               accum_out=dg[:, m:m + 1])
            dgs = small_p.tile([128, 1], F32, tag="dgs", name=f"dgs_{g}")
            nc.vector.tensor_add(dgs, dg[:, 0:1], dg[:, 1:2])
            trc = small_p.tile([128, 1], F32, tag="trc", name=f"trc_{g}")
            nc.gpsimd.partition_all_reduce(trc, dgs, 128, bass_isa.ReduceOp.add)
            rcpb = small_p.tile([128, 1], F32, tag="rcpb", name=f"rcpb_{g}")
            nc.vector.reciprocal(rcpb, trc)
            rcpn = small_p.tile([128, 1], F32, tag="rcpn", name=f"rcpn_{g}")
            nc.vector.tensor_scalar_mul(rcpn, rcpb, -0.5)
            cbb = small_p.tile([1, 1], F32, tag="cbb", name=f"cbb_{g}")
            nc.scalar.activation(cbb, trc[0:1, 0:1], ACTF.Sqrt, scale=1.0 / (NPG - 1))
            cb2 = small_p.tile([1, 1], F32, tag="cb2", name=f"cb2_{g}")
            nc.vector.tensor_scalar_mul(cb2, cbb, 1.0 / 8192.0)
            # V0' = 1.5I - 0.5*Gc/tr
            v0p = chain_p.tile([128, 512], BF16, tag="v0p", name=f"v0p_{g}")
            nc.vector.scalar_tensor_tensor(v0p, Gc, rcpn, C15, ALU.mult, ALU.add)
            # v0 row (all tail constants folded): cb2 = sqrt(tr/511)/8192
            v0r = small_p.tile([1, D], BF16, tag="v0r", name=f"v0r_{g}")
            nc.vector.tensor_scalar_mul(v0r, st["srow"], cb2)
            nc.sync.dma_start(
                v0rows[g // WAVE][32 * (g % WAVE):32 * (g % WAVE) + 1, :], v0r)
            st["Gc"], st["rcpb"], st["v0p"] = Gc, rcpb, v0p

        def mm256(dst_ps, L, R):
            for m in range(2):
                for k in range(2):
                    nc.tensor.matmul(dst_ps[:, 256 * m:256 * (m + 1)],
                                     L[:, 256 * k + 128 * m:256 * k + 128 * (m + 1)],
                                     R[:, 256 * k:256 * (k + 1)],
                                     start=(k == 0), stop=(k == 1))

        def s_q0(g):
            st = ST[g]
            ps = psC.tile([128, 512], F32, tag="C", name=f"psq0_{g}")
            mm256(ps, st["Gc"], st["v0p"])
            q0 = mats_p.tile([128, 512], BF16, tag="q0", name=f"q0_{g}")
            nc.scalar.activation(q0, ps, ACTF.Copy, scale=st["rcpb"])
            st["q0"] = q0

        def s_t1(g):
            st = ST[g]
            ps = psC.tile([128, 512], F32, tag="C", name=f"pst1_{g}")
            mm256(ps, st["q0"], st["v0p"])
            t1 = mats_p.tile([128, 512], BF16, tag="t1", name=f"t1_{g}")
            nc.scalar.copy(t1, ps)
            v1p = chain_p.tile([128, 512], BF16, tag="v1p", name=f"v1p_{g}")
            nc.vector.scalar_tensor_tensor(v1p, t1, -0.5, C15, ALU.mult, ALU.add)
            st["t1"], st["v1p"] = t1, v1p

        def s_q1(g):
            st = ST[g]
            ps = psC.tile([128, 512], F32, tag="C", name=f"psq1_{g}")
            mm256(ps, st["t1"], st["v1p"])
            qq = chain_p.tile([128, 512], BF16, tag="qq", name=f"qq_{g}")
            nc.scalar.copy(qq, ps)
            st["qq"] = qq

        def s_t2(g):
            st = ST[g]
            ps = psC.tile([128, 512], F32, tag="C", name=f"pst2_{g}")
            mm256(ps, st["qq"], st["v1p"])
            t2 = mats_p.tile([128, 512], BF16, tag="t2", name=f"t2_{g}")
            nc.scalar.copy(t2, ps)
            v2p = mats_p.tile([128, 512], BF16, tag="v2p", name=f"v2p_{g}")
            nc.vector.scalar_tensor_tensor(v2p, t2, -0.5, C15, ALU.mult, ALU.add)
            st["t2"], st["v2p"] = t2, v2p

        def s_q2(g):
            st = ST[g]
            ps = psC.tile([128, 512], F32, tag="C", name=f"psq2_{g}")
            mm256(ps, st["t2"], st["v2p"])
            qx2 = mats_p.tile([128, 512], BF16, tag="qx2", name=f"qx2_{g}")
            nc.scalar.copy(qx2, ps)
            st["qx2"] = qx2

        # ---- batched tail, tick-scheduled to overlap late phase-A ----
        # wave w = graphs 4w..4w+3; per step: 8 matvec matmuls land the 4
        # u-rows at quad partitions of one PSUM bank (part1), then a
        # selector transpose + combine produces the next v columns (part2).
        xkeys = ["T3", "T3", "T3", "T3", "t2", "t1", "q0"]
        kinds = ["comb", "comb", "a3", "comb", "comb", "comb", "final"]
        cur = {}
        v0c = {}
        TMP = {}

        def transpose_to_cols(src_sb, vc_ps):
            for m in range(2):
                nc.tensor.matmul(vc_ps[:, 4 * m:4 * m + 4],
                                 src_sb[:, 128 * m:128 * (m + 1)],
                                 E4, start=True, stop=True)

        def emit_v0c(w):
            def f():
                vc = psV.tile([128, 8], F32, tag="vc", name=f"v0vc_{w}")
                transpose_to_cols(v0rows[w], vc)
                v0 = vcol_p.tile([128, 8], BF16, tag="v0c", name=f"v0c_{w}")
                nc.scalar.copy(v0, vc)
                v0c[w] = v0
                cur[w] = v0
            return f

        def emit_p1(si, w):
            # Column-form matvec: stationary = X block [128,128], moving = v
            # column -> u chunks land directly as PSUM columns (no rows bank,
            # no transpose).  Final step stays row-form for the output DMA.
            def f():
                if kinds[si] == "final":
                    rows = psR.tile([128, 256], F32, tag="rows",
                                    name=f"rows_{si}_{w}")
                    for j in range(WAVE):
                        X = ST[WAVE * w + j][xkeys[si]]
                        for k in range(2):
                            nc.tensor.matmul(
                                rows[32 * j:32 * j + 1, :],
                                cur[w][:, 4 * k + j:4 * k + j + 1],
                                X[:, 256 * k:256 * (k + 1)],
                                start=(k == 0), stop=(k == 1),
                                tile_position=(0, 32 * j))
                    cf = tail_p.tile([128, 256], F32, tag="cf", name=f"cf_{w}")
                    nc.scalar.copy(cf, rows)
                    nc.sync.dma_start(out_d[WAVE * w:WAVE * (w + 1), :],
                                      cf[0:128:32, :])
                    return
                def colmv(vc_cols, Xkey, v_sb):
                    for j in range(WAVE):
                        X = ST[WAVE * w + j][Xkey]
                        for m in range(2):
                            for k in range(2):
                                nc.tensor.matmul(
                                    vc_cols[:, 4 * m + j:4 * m + j + 1],
                                    X[:, 256 * k + 128 * m:256 * k + 128 * (m + 1)],
                                    v_sb[:, 4 * k + j:4 * k + j + 1],
                                    start=(k == 0), stop=(k == 1))
                if xkeys[si] == "T3":
                    # T3 v == Q2 (V2' v): two chained column matvecs sharing
                    # one PSUM bank (w1 group closes before u group opens)
                    vc = psV.tile([128, 16], F32, tag="vc", name=f"vc_{si}_{w}")
                    colmv(vc[:, 0:8], "v2p", cur[w])
                    w1 = tail_p.tile([128, 8], BF16, tag="w1", name=f"w1_{si}_{w}")
                    nc.vector.tensor_scalar_mul(w1, vc[:, 0:8], 1.0)
                    colmv(vc[:, 8:16], "qx2", w1)
                    TMP[(si, w)] = vc[:, 8:16]
                else:
                    vc = psV.tile([128, 16], F32, tag="vc", name=f"vc_{si}_{w}")
                    colmv(vc[:, 0:8], xkeys[si], cur[w])
                    TMP[(si, w)] = vc[:, 0:8]
            return f

        def emit_p2(si, w):
            def f():
                vc = TMP.pop((si, w))
                vn = vcol_p.tile([128, 8], BF16, tag="vn", name=f"vn_{si}_{w}")
                if kinds[si] == "a3":
                    usb = tail_p.tile([128, 8], BF16, tag="usb",
                                      name=f"usb_{si}_{w}")
                    nc.scalar.mul(usb, vc, 0.25)
                    nc.vector.scalar_tensor_tensor(vn, v0c[w], 3.0, usb,
                                                   ALU.mult, ALU.subtract)
                else:
                    # combine straight from the matvec PSUM columns on DVE
                    nc.vector.scalar_tensor_tensor(vn, cur[w], 3.0, vc,
                                                   ALU.mult, ALU.subtract)
                cur[w] = vn
            return f

        from collections import defaultdict
        tail_sched = defaultdict(list)
        for w in range(N_WAVES):
            t0 = 12 + WAVE * w
            tail_sched[t0 - 1].append(emit_v0c(w))
            for si in range(7):
                tail_sched[t0 + si].append(emit_p1(si, w))
                if kinds[si] != "final":
                    tail_sched[t0 + si + 1].insert(0, emit_p2(si, w))

        # ---- phase A+chain: stage-skewed pipeline over the 16 graphs,
        #      with tail ticks interleaved ----
        stages = [s_q2, s_t2, s_q1, s_t1, s_q0, s_mid, s_gram, s_scol]
        n_st = len(stages)
        n_ticks = max(B_CORE + n_st + 1, max(tail_sched) + 1)
        for t in range(n_ticks):
            if t < B_CORE:
                s_load(t)
            for i, fn in enumerate(stages):
                g = t - (n_st - i)
                if 0 <= g < B_CORE:
                    fn(g)
            if t < B_CORE:
                s_gb(t)
            for f in tail_sched.get(t, ()):
                f()


_CACHED_NC = None


def _get_nc():
    global _CACHED_NC
    if _CACHED_NC is None:
        _CACHED_NC = build_module()
    return _CACHED_NC


def _run(feat, noise, **spmd_kwargs):
    feat = np.ascontiguousarray(np.asarray(feat), dtype=np.float32)
    noise = np.ascontiguousarray(np.asarray(noise), dtype=np.float32)
    cst = _const_arrays()
    nc = _get_nc()
    in_maps = []
    for c in range(N_CORES):
        in_maps.append({
            "feat": feat[c * ROWS_CORE:(c + 1) * ROWS_CORE],
            "noise": noise[c * ROWS_CORE:(c + 1) * ROWS_CORE],
            "cst": cst,
        })
    return run_bass_kernel_spmd(nc, in_maps, list(range(N_CORES)), **spmd_kwargs)


def kernel(feat, noise, n_per_graph):
    assert int(n_per_graph) == NPG
    try:
        res = _run(feat, noise)
    except Exception:
        # the axon device occasionally reports a transient unrecoverable
        # state; one retry usually succeeds
        res = _run(feat, noise)
    return np.concatenate([res.results[c]["out"] for c in range(N_CORES)], axis=0)
